# revision 54
# baseline (speedup 1.0000x reference)
"""CrossWinAttention Trainium2 Bass kernel.

Problem (hardcoded shapes): q/k/v (2,6,8,8,8,8,128) f32, windowed attention
over l=x*y=64 windows per batch, each window has T = n*w1*w2 = 384 tokens of
dim 128; LN -> QKV proj -> 4-head attention (dhead 32) -> out proj -> mean
over n agents -> + skip.

Sharding: the 2*64 = 128 (b, l) windows are fully independent -> 16 windows
per NeuronCore across 8 cores (SPMD: same program, per-core data).

v2 design (after perfetto analysis of v1: DVE 75% busy, input DMA spanning
the whole kernel at ~19 GB/s/queue over 2 queues):
  - Host packs q/k/v windows to bf16 in the exact device layout
    [win, p, i, k, d] (token t = 4p + k): halves DMA bytes, 3KB contiguous
    partition lines, no device-side rearrange. Loads round-robin over the
    3 available DMA queues (ACT + SP HWDGE, Pool SWDGE).
  - LN stats via single-group bn_stats (12/window, the HW op); even/odd
    halves combined in a few BATCHED [96, 16*12] DVE ops (small per-window
    DVE ops measured ~1-2us fixed overhead each on HW - avoid);
    rstd_raw = 1/sqrt(128*(var+eps)) via Ln+Exp on ACT (shared table set
    with the softmax Exp; sqrt(128) folded into host-side weights).
  - Normalize is FOLDED INTO the PE transpose: rhs D_w = [diag(rstd);
    -mu*rstd] ([97, 12, 96] bf16, diag built in ONE broadcast
    tensor_tensor per window), lhsT = raw bf16 x with an appended
    all-ones partition row. The transposed x^T comes out of PSUM already
    layer-normalized, eliminating all per-token normalize ops.
    The -mu*rstd rows come from 2 batch PE transposes of mr = -mu*rstd
    + one tiny SBUF->SBUF DMA per window into D_w's partition 96.
  - Softmax denominators via Ln+Exp on ACT (same table set as softmax
    Exp, no thrash) instead of the DVE's 8-cycle iterative divide.
    (reciprocal_approx_fast is not supported by this walrus build.)
  - PSUM evacuations split across ACT (x^T, 'copy' is in every ACT table
    set so no table thrash) and DVE (q/k/v).
Phase 2 per window (pipelined across windows, 5 stages):
  N(w): D_w diag build (DVE) + mu-row DMA
  A(w): transpose-normalize x^T + QKV projections (PE) + evacs (ACT/DVE)
  C(w): dot row-packed per kt into [128, 2*512] PSUM + Exp (ACT)
  D(w): denominators s + A^T=V^T exp (PE, col-packed) + 1/s (ACT Ln/Exp)
  E(w): out proj with wp/6 (mean over agents as strided DVE reduce),
        PE transpose, skip add, store
"""

import os
from contextlib import ExitStack

import numpy as np
import ml_dtypes

import concourse.bass as bass
import concourse.tile as tile
from concourse import mybir
from concourse.bass_utils import run_bass_kernel_spmd
from concourse.masks import make_identity

# ---- problem constants (must match the grading reference) ----
B, NAG, X, Y, W1, W2 = 2, 6, 8, 8, 8, 8
DIM, HEADS, DHEAD = 128, 4, 32
HD = HEADS * DHEAD
EPS = 1e-5
SCALE = DHEAD ** -0.5
N_CORES = 8
L = X * Y                    # 64 windows per batch
NWIN = B * L                 # 128 windows total
WPC = NWIN // N_CORES        # 16 windows per core
T = NAG * W1 * W2            # 384 tokens per window
TT = T // 128                # 3 token tiles
WTOK = W1 * W2               # 64 output tokens per window
KK = 4                       # consecutive tokens per SBUF partition row
TP = T // KK                 # 96 partitions used by phase-1 tiles
NG = 3 * KK                  # 12 (i,k) LN groups per window

F32 = mybir.dt.float32
BF16 = mybir.dt.bfloat16


def build_nc(n_win=WPC, qbias=False, kbias=False, iters=1):
    """Build the per-core Bass module.

    iters > 1 replays the whole body iters times (Python unroll) recomputing
    the same outputs; used only for wall-clock timing."""
    nc = bass.Bass(trn_type="TRN2")

    qkvi = nc.dram_tensor("qkvin", [n_win, 128, 3, KK, DIM], BF16, kind="ExternalInput")
    ski = nc.dram_tensor("skin", [n_win, WTOK, DIM], F32, kind="ExternalInput")
    wqd = nc.dram_tensor("wq", [DIM, HD], BF16, kind="ExternalInput")
    wkd = nc.dram_tensor("wk", [DIM, HD], BF16, kind="ExternalInput")
    wvd = nc.dram_tensor("wv", [DIM, HD], BF16, kind="ExternalInput")
    wpd = nc.dram_tensor("wp", [HD, DIM], BF16, kind="ExternalInput")
    bqd = nc.dram_tensor("bq", [1, HD], F32, kind="ExternalInput")
    bkd = nc.dram_tensor("bk", [1, HD], F32, kind="ExternalInput")
    outo = nc.dram_tensor("out", [n_win, WTOK, DIM], F32, kind="ExternalOutput")

    with tile.TileContext(nc) as tc, ExitStack() as ctx:
        consts = ctx.enter_context(tc.tile_pool(name="consts", bufs=1))
        pxin = ctx.enter_context(tc.tile_pool(name="pxin", bufs=1))
        pstat = ctx.enter_context(tc.tile_pool(name="pstat", bufs=1))
        pcmb = ctx.enter_context(tc.tile_pool(name="pcmb", bufs=1))
        pD = ctx.enter_context(tc.tile_pool(name="pD", bufs=4))
        pxt = ctx.enter_context(tc.tile_pool(name="pxt", bufs=3))
        pqkv = ctx.enter_context(tc.tile_pool(name="pqkv", bufs=4))
        pexp = ctx.enter_context(tc.tile_pool(name="pexp", bufs=3))
        prs = ctx.enter_context(tc.tile_pool(name="prs", bufs=3))
        pat = ctx.enter_context(tc.tile_pool(name="pat", bufs=3))
        ptail = ctx.enter_context(tc.tile_pool(name="ptail", bufs=3))
        pskip = ctx.enter_context(tc.tile_pool(name="pskip", bufs=6))
        # PSUM: 8 banks. pp (transp/proj/z+zt) 2 + dot 4 (4 heads) + s/av 2
        pp = ctx.enter_context(tc.tile_pool(name="pp", bufs=2, space="PSUM"))
        psdot = ctx.enter_context(tc.tile_pool(name="psdot", bufs=1, space="PSUM"))
        pssav = ctx.enter_context(tc.tile_pool(name="pssav", bufs=1, space="PSUM"))

        # ---- constants ----
        wq_sb = consts.tile([DIM, HD], BF16, tag="wq")
        wk_sb = consts.tile([DIM, HD], BF16, tag="wk")
        wv_sb = consts.tile([DIM, HD], BF16, tag="wv")
        wp_sb = consts.tile([HD, DIM], BF16, tag="wp")
        nc.scalar.dma_start(out=wq_sb, in_=wqd[:, :])
        nc.scalar.dma_start(out=wk_sb, in_=wkd[:, :])
        nc.scalar.dma_start(out=wv_sb, in_=wvd[:, :])
        nc.scalar.dma_start(out=wp_sb, in_=wpd[:, :])
        bq_sb = consts.tile([1, HD], F32, tag="bq")
        bk_sb = consts.tile([1, HD], F32, tag="bk")
        if qbias:
            nc.scalar.dma_start(out=bq_sb, in_=bqd[:, :])
        if kbias:
            nc.scalar.dma_start(out=bk_sb, in_=bkd[:, :])
        ones32 = consts.tile([128, 32], BF16, tag="ones32")
        nc.vector.memset(ones32, 1.0)
        ones1 = consts.tile([1, T], BF16, tag="ones1")
        nc.vector.memset(ones1, 1.0)
        eps_t = consts.tile([TP, 1], F32, tag="eps")
        nc.vector.memset(eps_t, 128.0 * EPS)
        ident96 = consts.tile([TP, TP], BF16, tag="ident96")
        make_identity(nc, ident96[:, :])
        identf = consts.tile([128, 128], F32, tag="identf")
        make_identity(nc, identf[:, :])
        warm_sb = consts.tile([128, 512], BF16, tag="warm")
        nc.vector.memset(warm_sb, 0.5)

        # input loads round-robin over the three DMA queues
        dmaq = [nc.scalar, nc.sync, nc.gpsimd]

        # PE prewarm: HAM un-throttles (1.2 -> 2.4 GHz) only after ~3.4us of
        # sustained matmul activity; phase 1 is PE-idle, so without this the
        # whole pipeline starts (and tends to stay) at half clock. The dummy
        # N=512 matmuls keep the PE busy from t~1us through the pipeline
        # ramp-up so real work starts (and stays) at full clock.
        for _ in range(30):
            warm_ps = pp.tile([128, 512], F32, tag="pj")
            nc.tensor.matmul(
                warm_ps, lhsT=warm_sb[:, 0:128], rhs=warm_sb,
                start=True, stop=True,
            )

        for _it in range(iters):
            # ---- phase 1: loads + grouped LN stats ----
            # Each of the TP=96 partitions holds KK=4 consecutive tokens per
            # operand; partition line = 3KB contiguous DRAM (host pre-packed).
            # Token t lives at (p = t//4, k = t%4); agent n = p//16.
            stats = pstat.tile([TP, n_win, NG, 6], F32, tag="stats")
            wph = 4                     # windows per combine group (4 groups:
            nhalf = n_win // wph        # phase 2 starts after only 4 loads)
            mrT_sb = pcmb.tile([wph * NG, nhalf, TP], BF16, tag="mrT")
            x_w = []
            for w in range(n_win):
                # row 96 = 1.0 and rows 97..127 = 0, packed host-side
                xw = pxin.tile([128, 3, KK, DIM], BF16, tag=f"x{w}")
                x_w.append(xw)
                dmaq[w % 3].dma_start(out=xw, in_=qkvi[w])

            def phase1_stats(w):
                # HW BNStats is single-group: one op per (i, k) token block
                for i in range(3):
                    for k in range(KK):
                        nc.vector.bn_stats(
                            out=stats[:, w, KK * i + k, :],
                            in_=x_w[w][0:TP, i, k, :],
                        )

            rstd_b = {}

            def combine_half(h):
                # combine bn_stats' even/odd halves for windows of half h:
                # mu = (m_e+m_o)/2, var128 = 128*var = cv_e+cv_o+32*(m_e-m_o)^2
                st = stats[:, wph * h : wph * (h + 1), :, :]
                m_e, m_o = st[:, :, :, 1], st[:, :, :, 4]
                cv_e, cv_o = st[:, :, :, 2], st[:, :, :, 5]
                dmm = pcmb.tile([TP, wph, NG], F32, tag=f"dmm{h}")
                nc.vector.tensor_sub(dmm, m_e, m_o)
                nc.vector.tensor_mul(dmm, dmm, dmm)
                var128 = pcmb.tile([TP, wph, NG], F32, tag=f"var{h}")
                nc.vector.tensor_add(var128, cv_e, cv_o)
                nc.vector.scalar_tensor_tensor(
                    out=var128, in0=dmm, scalar=32.0, in1=var128,
                    op0=mybir.AluOpType.mult, op1=mybir.AluOpType.add,
                )
                musum = pcmb.tile([TP, wph, NG], F32, tag=f"musum{h}")
                nc.vector.tensor_add(musum, m_e, m_o)
                # rstd_raw = (var128 + 128eps)^-0.5 via Ln+Exp (exp table set)
                nc.scalar.activation(
                    out=var128, in_=var128,
                    func=mybir.ActivationFunctionType.Ln, bias=eps_t,
                )
                rstd = pcmb.tile([TP, wph, NG], F32, tag=f"rstd{h}")
                nc.scalar.activation(
                    out=rstd, in_=var128,
                    func=mybir.ActivationFunctionType.Exp, scale=-0.5,
                )
                rb = pcmb.tile([TP, wph, NG], BF16, tag=f"rstdb{h}")
                nc.vector.tensor_copy(rb, rstd)
                rstd_b[h] = rb
                # mr = -mu*rstd (bf16), transposed so row (w%8)*12+j holds
                # the 96 token values of window w, block j
                mr = pcmb.tile([TP, wph, NG], BF16, tag=f"mr{h}")
                nc.vector.tensor_mul(mr, musum, rstd)
                nc.vector.tensor_scalar(
                    out=mr, in0=mr, scalar1=-0.5, scalar2=None,
                    op0=mybir.AluOpType.mult,
                )
                mrt_ps = pssav.tile([wph * NG, TP], BF16, tag="s")
                nc.tensor.transpose(
                    out=mrt_ps,
                    in_=mr.rearrange("p a b -> p (a b)"),
                    identity=ident96[:, :],
                )
                nc.vector.tensor_copy(mrT_sb[:, h, :], mrt_ps)

            # ---- pipelined per-window stages ----
            D_w, skip_w, qT_w, kT_w, vh_w, expT_w, aT_w = {}, {}, {}, {}, {}, {}, {}

            def stage_n(w):
                # D_w[p, j, c] = delta(p, c) * rstd[p, w, j] for p < 96;
                # row 96 = -mu*rstd (via DMA from the batch transpose)
                Dw = pD.tile([TP + 1, NG, TP], BF16, tag="D")
                nc.vector.tensor_mul(
                    Dw[0:TP, :, :],
                    ident96[:, :].unsqueeze(1).broadcast_to([TP, NG, TP]),
                    rstd_b[w // wph][:, w % wph, :]
                    .unsqueeze(2)
                    .broadcast_to([TP, NG, TP]),
                )
                nc.sync.dma_start(
                    out=Dw[TP : TP + 1, :, :],
                    in_=mrT_sb[NG * (w % wph) : NG * (w % wph) + NG, w // wph, :],
                )
                D_w[w] = Dw

            def stage_a(w):
                # skip rows in device token order (k, r); host pre-permuted
                skip_sb = pskip.tile([WTOK, DIM], F32, tag="skip")
                nc.sync.dma_start(out=skip_sb, in_=ski[w])
                skip_w[w] = skip_sb
                xw, Dw = x_w[w], D_w[w]
                # transpose-normalize to [d, t'] on PE (raw bf16 x stationary,
                # D_w moving): x^T column c = k*96 + p <-> token t = 4p + k
                xT_sb = pxt.tile([128, 3, T], BF16, tag="xT")
                for i in range(3):
                    tp = pp.tile([128, 512], F32, tag="pj")
                    for k in range(KK):
                        nc.tensor.matmul(
                            tp[:, k * TP : (k + 1) * TP],
                            lhsT=xw[0 : TP + 1, i, k, :],
                            rhs=Dw[:, KK * i + k, :],
                            start=True, stop=True,
                        )
                    nc.scalar.copy(out=xT_sb[:, i, :], in_=tp[:, 0:T])
                # projections: q, k -> [hd, t]
                qT_sb = pqkv.tile([HD, T], BF16, tag="qT")
                kT_sb = pqkv.tile([HD, T], BF16, tag="kT")
                for i, (w_sb, b_sb, has_b, dst) in enumerate(
                    ((wq_sb, bq_sb, qbias, qT_sb), (wk_sb, bk_sb, kbias, kT_sb))
                ):
                    ppj = pp.tile([HD, T], F32, tag="pj")
                    nc.tensor.matmul(
                        ppj, lhsT=w_sb, rhs=xT_sb[:, i, :], start=True, stop=True
                    )
                    if has_b:
                        nc.tensor.matmul(
                            ppj, lhsT=b_sb, rhs=ones1, start=False, stop=True,
                            skip_group_check=True,
                        )
                    nc.vector.tensor_copy(dst, ppj)
                # v -> [t, hd] (token-major, the AV stationary operand)
                pv = pp.tile([128, TT, HD], F32, tag="pj")
                for j in range(TT):
                    nc.tensor.matmul(
                        pv[:, j, :],
                        lhsT=xT_sb[:, 2, j * 128 : (j + 1) * 128],
                        rhs=wv_sb, start=True, stop=True,
                    )
                vh_sb = pqkv.tile([128, TT, HD], BF16, tag="vh")
                nc.vector.tensor_copy(vh_sb, pv)
                qT_w[w], kT_w[w], vh_w[w] = qT_sb, kT_sb, vh_sb

            def stage_cd(cw, dw):
                # Interleaved emission: window cw's dot+exp (stage C) with
                # window dw = cw-1's s/av (stage D). The dot PSUM has one
                # 4-bank ring slot, so dot(kt+1) blocks the strict PE FIFO
                # until exp(kt) drains it; the s/av matmuls of the previous
                # window slotted between dot groups keep the PE fed through
                # those waits (and the HAM activity monitor warm).
                if cw is not None:
                    qT_sb, kT_sb = qT_w[cw], kT_w[cw]
                    expT_sb = pexp.tile([128, TT, HEADS, T], BF16, tag="expT")
                    expT_w[cw] = expT_sb
                if dw is not None:
                    expT_d, vh_sb = expT_w[dw], vh_w[dw]
                    s_ps = pssav.tile([HD, T], F32, tag="s")
                    av_ps = pssav.tile([HD, T], F32, tag="av")
                for kt in range(TT):
                    if cw is not None:
                        # 4 heads row-tiled (rows 32h) -> concurrent on PE;
                        # ONE Exp per kt (512-col spacing keeps each head's
                        # output inside one PSUM bank)
                        dt = psdot.tile([128, HEADS, 512], F32, tag="dot")
                        for h in range(HEADS):
                            nc.tensor.matmul(
                                dt[:, h, 0:T],
                                lhsT=kT_sb[32 * h : 32 * (h + 1), kt * 128 : (kt + 1) * 128],
                                rhs=qT_sb[32 * h : 32 * (h + 1), :],
                                start=True, stop=True,
                                tile_position=(32 * h, 0),
                            )
                        nc.scalar.activation(
                            out=expT_sb[:, kt, :, :],
                            in_=dt[:, :, 0:T],
                            func=mybir.ActivationFunctionType.Exp,
                        )
                    if dw is not None:
                        # 4 heads' s then av back-to-back: distinct col
                        # groups run concurrently (~2.4x measured for 3-col)
                        for h in range(HEADS):
                            nc.tensor.matmul(
                                s_ps[32 * h : 32 * (h + 1), :],
                                lhsT=ones32, rhs=expT_d[:, kt, h, :],
                                start=(kt == 0), stop=(kt == TT - 1),
                                tile_position=(0, 32 * h), skip_group_check=True,
                            )
                        for h in range(HEADS):
                            nc.tensor.matmul(
                                av_ps[32 * h : 32 * (h + 1), :],
                                lhsT=vh_sb[:, kt, 32 * h : 32 * (h + 1)],
                                rhs=expT_d[:, kt, h, :],
                                start=(kt == 0), stop=(kt == TT - 1),
                                tile_position=(0, 32 * h), skip_group_check=True,
                            )
                if dw is not None:
                    # 1/s = Exp(-Ln(s)) on ACT (both in the softmax-Exp
                    # table set, no thrash; frees the DVE's 8-cycle divide)
                    ls_sb = prs.tile([HD, T], F32, tag="ls")
                    nc.scalar.activation(
                        out=ls_sb, in_=s_ps,
                        func=mybir.ActivationFunctionType.Ln,
                    )
                    rs_sb = prs.tile([HD, T], F32, tag="rs")
                    nc.scalar.activation(
                        out=rs_sb, in_=ls_sb,
                        func=mybir.ActivationFunctionType.Exp, scale=-1.0,
                    )
                    aT_sb = pat.tile([HD, T], BF16, tag="aT")
                    nc.vector.tensor_mul(aT_sb, av_ps, rs_sb)
                    aT_w[dw] = aT_sb

            def stage_e(w):
                aT_sb = aT_w[w]
                # out proj all 6 agents in one matmul; mean over agents as a
                # strided DVE reduce: q col c = k*96 + n*16 + r (u = 4r + k).
                # z occupies cols 0..383 of a pj slot; the transposed result
                # is carved into the same bank at cols 384..511.
                # (moving the agent-mean BEFORE the z matmul was tried and
                # measured 5.6us SLOWER: the reduce lands on the aT->z
                # critical path instead of overlapping downstream work.)
                zz = pp.tile([DIM, 512], F32, tag="pj")
                nc.tensor.matmul(
                    zz[:, 0:T], lhsT=wp_sb, rhs=aT_sb, start=True, stop=True
                )
                zT_sb = ptail.tile([DIM, WTOK], F32, tag="zT")
                nc.vector.tensor_reduce(
                    out=zT_sb.rearrange("p (k r) -> p k r", k=KK),
                    in_=zz[:, 0:T].rearrange("p (k n r) -> p k r n", k=KK, n=NAG),
                    axis=mybir.AxisListType.X,
                    op=mybir.AluOpType.add,
                )
                nc.tensor.transpose(
                    out=zz[0:WTOK, T : T + DIM], in_=zT_sb,
                    identity=identf[:, :],
                )
                out_sb = ptail.tile([WTOK, DIM], F32, tag="osb")
                nc.vector.tensor_add(out_sb, zz[0:WTOK, T : T + DIM], skip_w[w])
                # store in device partition order; host un-permutes
                nc.sync.dma_start(out=outo[w], in_=out_sb)

            # stats for the first half up front, second half interleaved into
            # the pipeline (keeps the DVE FIFO free of not-yet-loaded waits)
            for w in range(wph):
                phase1_stats(w)
            combine_half(0)
            for step in range(n_win + 4):
                if wph + step < n_win:
                    phase1_stats(wph + step)
                if 0 < step < n_win and step % wph == 0:
                    combine_half(step // wph)
                if step < n_win:
                    stage_n(step)
                if 0 <= step - 1 < n_win:
                    stage_a(step - 1)
                cw = step - 2 if 0 <= step - 2 < n_win else None
                dw = step - 3 if 0 <= step - 3 < n_win else None
                if cw is not None or dw is not None:
                    stage_cd(cw, dw)
                if 0 <= step - 4 < n_win:
                    stage_e(step - 4)

    return nc


def _split_multiwaits(nc, limit=1):
    """The staged walrus build rejects instructions carrying more than one
    sync-wait condition. Tile attaches several to some instructions (and the
    kernel-tail drain); peel the extras onto preceding engine NoOps. HW-only:
    CoreSim's sem bookkeeping rejects the injected NoOps."""
    for f in nc.m.functions:
        for bb in f.blocks:
            new_list = []
            for inst in bb.instructions:
                si = getattr(inst, "sync_info", None)
                waits = list(si.on_wait) if si is not None and si.on_wait else []
                if len(waits) > limit:
                    extra, keep = waits[:-limit], waits[-limit:]
                    for j in range(0, len(extra), limit):
                        nop = mybir.InstNoOp(
                            name=nc.get_next_instruction_name(),
                            engine=inst.engine,
                            ins=[],
                            outs=[],
                            sync_info=mybir.SyncInfo(
                                on_wait=extra[j : j + limit], on_update=[]
                            ),
                        )
                        new_list.append(nop)
                    si.on_wait = keep
                new_list.append(inst)
            if len(new_list) != len(bb.instructions):
                bb.instructions = new_list
    return nc


def _prep(inputs):
    """Host-side constant folding + window gather + device-layout pack +
    shard. Returns (in_maps, qbias, kbias)."""
    f32 = np.float32
    q = np.asarray(inputs["q"], f32)
    k = np.asarray(inputs["k"], f32)
    v = np.asarray(inputs["v"], f32)
    skip = np.asarray(inputs["skip"], f32)
    gate = np.asarray(inputs["head_gate"], f32)
    lnqw, lnqb = np.asarray(inputs["ln_q_w"], f32), np.asarray(inputs["ln_q_b"], f32)
    lnkw, lnkb = np.asarray(inputs["ln_k_w"], f32), np.asarray(inputs["ln_k_b"], f32)
    lnvw, lnvb = np.asarray(inputs["ln_v_w"], f32), np.asarray(inputs["ln_v_b"], f32)
    wq, bq = np.asarray(inputs["wq"], f32), np.asarray(inputs["bq"], f32)
    wk, bk = np.asarray(inputs["wk"], f32), np.asarray(inputs["bk"], f32)
    wv, bv = np.asarray(inputs["wv"], f32), np.asarray(inputs["bv"], f32)
    wp, bp = np.asarray(inputs["wp"], f32), np.asarray(inputs["bp"], f32)

    # fold LN affine into the projections; fold softmax scale + head_gate
    # into the q side (dot*gate == (qh*gate).kh); fold sqrt(DIM) into all
    # three (device rstd_raw = rstd_true/sqrt(DIM))
    rdim = np.sqrt(np.float32(DIM))
    colscale = np.repeat(gate * SCALE, DHEAD)          # [HD]
    wq_f = (lnqw[:, None] * wq) * colscale[None, :] * rdim
    bq_f = lnqb @ wq * colscale + bq * colscale
    wk_f = lnkw[:, None] * wk * rdim
    bk_f = lnkb @ wk + bk
    wv_f = lnvw[:, None] * wv * rdim
    bv_f = lnvb @ wv + bv
    wp_f = wp / NAG
    # constant v offset passes straight through attention (softmax sums to 1)
    skip_c = bv_f @ wp + bp                             # [DIM]

    qbias = bool(np.any(bq_f != 0))
    kbias = bool(np.any(bk_f != 0))

    def windows(t):
        return t.transpose(0, 2, 3, 1, 4, 5, 6).reshape(NWIN, T, DIM)

    bf = ml_dtypes.bfloat16
    qkvw = np.stack([windows(q), windows(k), windows(v)], axis=1)
    # device layout [win, p, i, k, d]: token t = 4p + k, 3KB partition lines.
    # Rows 96..127 padded host-side: row 96 = 1.0 (the -mu*rstd matmul row),
    # rows 97..127 = 0 — no device-side memsets needed.
    qkv_dev = np.zeros((NWIN, 128, 3, KK, DIM), dtype=bf)
    qkv_dev[:, :TP] = (
        qkvw.reshape(NWIN, 3, TP, KK, DIM).transpose(0, 2, 1, 3, 4).astype(bf)
    )
    qkv_dev[:, TP] = np.ones((3, KK, DIM), dtype=bf)
    qkv_dev = np.ascontiguousarray(qkv_dev)
    skw = (skip + skip_c).reshape(NWIN, WTOK, DIM)
    # device partition order (k, r) for output token u = 4r + k
    sk_dev = np.ascontiguousarray(
        skw.reshape(NWIN, WTOK // KK, KK, DIM).transpose(0, 2, 1, 3)
        .reshape(NWIN, WTOK, DIM)
    )

    wq_b = np.ascontiguousarray(wq_f.astype(bf))
    wk_b = np.ascontiguousarray(wk_f.astype(bf))
    wv_b = np.ascontiguousarray(wv_f.astype(bf))
    wp_b = np.ascontiguousarray(wp_f.astype(bf))

    in_maps = []
    for c in range(N_CORES):
        sl = slice(c * WPC, (c + 1) * WPC)
        in_maps.append(
            {
                "qkvin": qkv_dev[sl],
                "skin": sk_dev[sl],
                "wq": wq_b,
                "wk": wk_b,
                "wv": wv_b,
                "wp": wp_b,
                "bq": np.ascontiguousarray(bq_f[None, :]),
                "bk": np.ascontiguousarray(bk_f[None, :]),
            }
        )
    return in_maps, qbias, kbias


_BUILD_CACHE = {}


def _trace_available():
    try:
        from antenv.axon_hooks import get_axon_ntff_profile_hook  # noqa: F401

        return get_axon_ntff_profile_hook() is not None
    except Exception:
        return False


def run_sharded(in_maps, qbias, kbias, iters=1, trace=False):
    key = (qbias, kbias, iters)
    if key not in _BUILD_CACHE:
        # wait-splitting is for the walrus compiler only; CoreSim paths use
        # build_nc directly without it
        _BUILD_CACHE[key] = _split_multiwaits(
            build_nc(WPC, qbias=qbias, kbias=kbias, iters=iters)
        )
    nc = _BUILD_CACHE[key]
    return run_bass_kernel_spmd(
        nc, in_maps, core_ids=list(range(N_CORES)), trace=trace,
    )


def kernel(**inputs) -> np.ndarray:
    in_maps, qbias, kbias = _prep(inputs)
    trace = bool(int(os.environ.get("KERNEL_TRACE", "0"))) and _trace_available()
    res = run_sharded(in_maps, qbias, kbias, iters=1, trace=trace)
    if trace and res.exec_time_ns is not None:
        kernel.last_exec_time_ns = res.exec_time_ns
        kernel.last_trace = res.instructions_and_trace
    out = np.concatenate([r["out"] for r in res.results], axis=0)  # [128,64,128]
    # device partition order (k, r) -> token order u = 4r + k
    out = (
        out.reshape(NWIN, KK, WTOK // KK, DIM)
        .transpose(0, 2, 1, 3)
        .reshape(NWIN, WTOK, DIM)
    )
    out = out.reshape(B, X, Y, W1, W2, DIM)
    return np.ascontiguousarray(out.astype(np.float32))


# revision 56
# speedup vs baseline: 1.1314x; 1.1314x over previous
"""CrossWinAttention Trainium2 Bass kernel.

Problem (hardcoded shapes): q/k/v (2,6,8,8,8,8,128) f32, windowed attention
over l=x*y=64 windows per batch, each window has T = n*w1*w2 = 384 tokens of
dim 128; LN -> QKV proj -> 4-head attention (dhead 32) -> out proj -> mean
over n agents -> + skip.

Sharding: the 2*64 = 128 (b, l) windows are fully independent -> 16 windows
per NeuronCore across 8 cores (SPMD: same program, per-core data).

v2 design (after perfetto analysis of v1: DVE 75% busy, input DMA spanning
the whole kernel at ~19 GB/s/queue over 2 queues):
  - Host packs q/k/v windows to bf16 in the exact device layout
    [win, p, i, k, d] (token t = 4p + k): halves DMA bytes, 3KB contiguous
    partition lines, no device-side rearrange. Loads round-robin over the
    3 available DMA queues (ACT + SP HWDGE, Pool SWDGE).
  - LN stats via single-group bn_stats (12/window, the HW op); even/odd
    halves combined in a few BATCHED [96, 16*12] DVE ops (small per-window
    DVE ops measured ~1-2us fixed overhead each on HW - avoid);
    rstd_raw = 1/sqrt(128*(var+eps)) via Ln+Exp on ACT (shared table set
    with the softmax Exp; sqrt(128) folded into host-side weights).
  - Normalize is FOLDED INTO the PE transpose: rhs D_w = [diag(rstd);
    -mu*rstd] ([97, 12, 96] bf16, diag built in ONE broadcast
    tensor_tensor per window), lhsT = raw bf16 x with an appended
    all-ones partition row. The transposed x^T comes out of PSUM already
    layer-normalized, eliminating all per-token normalize ops.
    The -mu*rstd rows come from 2 batch PE transposes of mr = -mu*rstd
    + one tiny SBUF->SBUF DMA per window into D_w's partition 96.
  - Softmax denominators via Ln+Exp on ACT (same table set as softmax
    Exp, no thrash) instead of the DVE's 8-cycle iterative divide.
    (reciprocal_approx_fast is not supported by this walrus build.)
  - PSUM evacuations split across ACT (x^T, 'copy' is in every ACT table
    set so no table thrash) and DVE (q/k/v).
Phase 2 per window (pipelined across windows, 5 stages):
  N(w): D_w diag build (DVE) + mu-row DMA
  A(w): transpose-normalize x^T + QKV projections (PE) + evacs (ACT/DVE)
  C(w): dot row-packed per kt into [128, 2*512] PSUM + Exp (ACT)
  D(w): denominators s + A^T=V^T exp (PE, col-packed) + 1/s (ACT Ln/Exp)
  E(w): out proj with wp/6 (mean over agents as strided DVE reduce),
        PE transpose, skip add, store
"""

import os
from contextlib import ExitStack

import numpy as np
import ml_dtypes

import concourse.bass as bass
import concourse.tile as tile
from concourse import mybir
from concourse.bass_utils import run_bass_kernel_spmd
from concourse.masks import make_identity

# ---- problem constants (must match the grading reference) ----
B, NAG, X, Y, W1, W2 = 2, 6, 8, 8, 8, 8
DIM, HEADS, DHEAD = 128, 4, 32
HD = HEADS * DHEAD
EPS = 1e-5
SCALE = DHEAD ** -0.5
N_CORES = 8
L = X * Y                    # 64 windows per batch
NWIN = B * L                 # 128 windows total
WPC = NWIN // N_CORES        # 16 windows per core
T = NAG * W1 * W2            # 384 tokens per window
TT = T // 128                # 3 token tiles
WTOK = W1 * W2               # 64 output tokens per window
KK = 4                       # consecutive tokens per SBUF partition row
TP = T // KK                 # 96 partitions used by phase-1 tiles
NG = 3 * KK                  # 12 (i,k) LN groups per window

F32 = mybir.dt.float32
BF16 = mybir.dt.bfloat16


def build_nc(n_win=WPC, qbias=False, kbias=False, iters=1):
    """Build the per-core Bass module.

    iters > 1 replays the whole body iters times (Python unroll) recomputing
    the same outputs; used only for wall-clock timing."""
    nc = bass.Bass(trn_type="TRN2")

    qkvi = nc.dram_tensor("qkvin", [n_win, 128, 3, KK, DIM], BF16, kind="ExternalInput")
    ski = nc.dram_tensor("skin", [n_win, WTOK, DIM], F32, kind="ExternalInput")
    wqd = nc.dram_tensor("wq", [DIM, HD], BF16, kind="ExternalInput")
    wkd = nc.dram_tensor("wk", [DIM, HD], BF16, kind="ExternalInput")
    wvd = nc.dram_tensor("wv", [DIM, HD], BF16, kind="ExternalInput")
    wpd = nc.dram_tensor("wp", [HD, DIM], BF16, kind="ExternalInput")
    bqd = nc.dram_tensor("bq", [1, HD], F32, kind="ExternalInput")
    bkd = nc.dram_tensor("bk", [1, HD], F32, kind="ExternalInput")
    outo = nc.dram_tensor("out", [n_win, WTOK, DIM], F32, kind="ExternalOutput")

    with tile.TileContext(nc) as tc, ExitStack() as ctx:
        consts = ctx.enter_context(tc.tile_pool(name="consts", bufs=1))
        pxin = ctx.enter_context(tc.tile_pool(name="pxin", bufs=1))
        pstat = ctx.enter_context(tc.tile_pool(name="pstat", bufs=1))
        pcmb = ctx.enter_context(tc.tile_pool(name="pcmb", bufs=1))
        pD = ctx.enter_context(tc.tile_pool(name="pD", bufs=4))
        pxt = ctx.enter_context(tc.tile_pool(name="pxt", bufs=3))
        pqkv = ctx.enter_context(tc.tile_pool(name="pqkv", bufs=4))
        pexp = ctx.enter_context(tc.tile_pool(name="pexp", bufs=3))
        prs = ctx.enter_context(tc.tile_pool(name="prs", bufs=3))
        pat = ctx.enter_context(tc.tile_pool(name="pat", bufs=3))
        ptail = ctx.enter_context(tc.tile_pool(name="ptail", bufs=3))
        pskip = ctx.enter_context(tc.tile_pool(name="pskip", bufs=6))
        # PSUM: 8 banks. pp (transp/proj/z+zt) 2 + dot 4 (4 heads) + s/av 2
        pp = ctx.enter_context(tc.tile_pool(name="pp", bufs=2, space="PSUM"))
        psdot = ctx.enter_context(tc.tile_pool(name="psdot", bufs=1, space="PSUM"))
        pssav = ctx.enter_context(tc.tile_pool(name="pssav", bufs=1, space="PSUM"))

        # ---- constants ----
        wq_sb = consts.tile([DIM, HD], BF16, tag="wq")
        wk_sb = consts.tile([DIM, HD], BF16, tag="wk")
        wv_sb = consts.tile([DIM, HD], BF16, tag="wv")
        wp_sb = consts.tile([HD, DIM], BF16, tag="wp")
        nc.scalar.dma_start(out=wq_sb, in_=wqd[:, :])
        nc.scalar.dma_start(out=wk_sb, in_=wkd[:, :])
        nc.scalar.dma_start(out=wv_sb, in_=wvd[:, :])
        nc.scalar.dma_start(out=wp_sb, in_=wpd[:, :])
        bq_sb = consts.tile([1, HD], F32, tag="bq")
        bk_sb = consts.tile([1, HD], F32, tag="bk")
        if qbias:
            nc.scalar.dma_start(out=bq_sb, in_=bqd[:, :])
        if kbias:
            nc.scalar.dma_start(out=bk_sb, in_=bkd[:, :])
        ones32 = consts.tile([128, 32], BF16, tag="ones32")
        nc.vector.memset(ones32, 1.0)
        ones1 = consts.tile([1, T], BF16, tag="ones1")
        nc.vector.memset(ones1, 1.0)
        eps_t = consts.tile([TP, 1], F32, tag="eps")
        nc.vector.memset(eps_t, 128.0 * EPS)
        ident96 = consts.tile([TP, TP], BF16, tag="ident96")
        make_identity(nc, ident96[:, :])
        identf = consts.tile([128, 128], F32, tag="identf")
        make_identity(nc, identf[:, :])
        warm_sb = consts.tile([128, 512], BF16, tag="warm")
        nc.vector.memset(warm_sb, 0.5)

        # input loads round-robin over the three DMA queues
        dmaq = [nc.scalar, nc.sync, nc.gpsimd]

        # PE prewarm: HAM un-throttles (1.2 -> 2.4 GHz) only after ~3.4us of
        # sustained matmul activity; phase 1 is PE-idle, so without this the
        # whole pipeline starts (and tends to stay) at half clock. The dummy
        # N=512 matmuls keep the PE busy from t~1us through the pipeline
        # ramp-up so real work starts (and stays) at full clock.
        for _ in range(36):
            warm_ps = pp.tile([128, 512], F32, tag="pj")
            nc.tensor.matmul(
                warm_ps, lhsT=warm_sb[:, 0:128], rhs=warm_sb,
                start=True, stop=True,
            )

        for _it in range(iters):
            # ---- phase 1: loads + grouped LN stats ----
            # Each of the TP=96 partitions holds KK=4 consecutive tokens per
            # operand; partition line = 3KB contiguous DRAM (host pre-packed).
            # Token t lives at (p = t//4, k = t%4); agent n = p//16.
            stats = pstat.tile([TP, n_win, NG, 6], F32, tag="stats")
            wph = TP // NG              # 8 windows per combine half
            nhalf = n_win // wph        # 2 halves
            mrT_sb = pcmb.tile([TP, nhalf, TP], BF16, tag="mrT")
            x_w = []
            for w in range(n_win):
                # row 96 = 1.0 and rows 97..127 = 0, packed host-side
                xw = pxin.tile([128, 3, KK, DIM], BF16, tag=f"x{w}")
                x_w.append(xw)
                dmaq[w % 3].dma_start(out=xw, in_=qkvi[w])

            def phase1_stats(w):
                # HW BNStats is single-group: one op per (i, k) token block
                for i in range(3):
                    for k in range(KK):
                        nc.vector.bn_stats(
                            out=stats[:, w, KK * i + k, :],
                            in_=x_w[w][0:TP, i, k, :],
                        )

            rstd_b = {}

            def combine_half(h):
                # combine bn_stats' even/odd halves for windows of half h:
                # mu = (m_e+m_o)/2, var128 = 128*var = cv_e+cv_o+32*(m_e-m_o)^2
                st = stats[:, wph * h : wph * (h + 1), :, :]
                m_e, m_o = st[:, :, :, 1], st[:, :, :, 4]
                cv_e, cv_o = st[:, :, :, 2], st[:, :, :, 5]
                dmm = pcmb.tile([TP, wph, NG], F32, tag=f"dmm{h}")
                nc.vector.tensor_sub(dmm, m_e, m_o)
                nc.vector.tensor_mul(dmm, dmm, dmm)
                var128 = pcmb.tile([TP, wph, NG], F32, tag=f"var{h}")
                nc.vector.tensor_add(var128, cv_e, cv_o)
                nc.vector.scalar_tensor_tensor(
                    out=var128, in0=dmm, scalar=32.0, in1=var128,
                    op0=mybir.AluOpType.mult, op1=mybir.AluOpType.add,
                )
                musum = pcmb.tile([TP, wph, NG], F32, tag=f"musum{h}")
                nc.vector.tensor_add(musum, m_e, m_o)
                # rstd_raw = (var128 + 128eps)^-0.5 via Ln+Exp (exp table set)
                nc.scalar.activation(
                    out=var128, in_=var128,
                    func=mybir.ActivationFunctionType.Ln, bias=eps_t,
                )
                rstd = pcmb.tile([TP, wph, NG], F32, tag=f"rstd{h}")
                nc.scalar.activation(
                    out=rstd, in_=var128,
                    func=mybir.ActivationFunctionType.Exp, scale=-0.5,
                )
                rb = pcmb.tile([TP, wph, NG], BF16, tag=f"rstdb{h}")
                nc.vector.tensor_copy(rb, rstd)
                rstd_b[h] = rb
                # mr = -mu*rstd (bf16), transposed so row (w%8)*12+j holds
                # the 96 token values of window w, block j
                mr = pcmb.tile([TP, wph, NG], BF16, tag=f"mr{h}")
                nc.vector.tensor_mul(mr, musum, rstd)
                nc.vector.tensor_scalar(
                    out=mr, in0=mr, scalar1=-0.5, scalar2=None,
                    op0=mybir.AluOpType.mult,
                )
                mrt_ps = pssav.tile([TP, TP], BF16, tag="s")
                nc.tensor.transpose(
                    out=mrt_ps,
                    in_=mr.rearrange("p a b -> p (a b)"),
                    identity=ident96[:, :],
                )
                nc.vector.tensor_copy(mrT_sb[:, h, :], mrt_ps)

            # ---- pipelined per-window stages ----
            D_w, skip_w, qT_w, kT_w, vh_w, expT_w, aT_w = {}, {}, {}, {}, {}, {}, {}

            def stage_n(w):
                # D_w[p, j, c] = delta(p, c) * rstd[p, w, j] for p < 96;
                # row 96 = -mu*rstd (via DMA from the batch transpose)
                Dw = pD.tile([TP + 1, NG, TP], BF16, tag="D")
                nc.vector.tensor_mul(
                    Dw[0:TP, :, :],
                    ident96[:, :].unsqueeze(1).broadcast_to([TP, NG, TP]),
                    rstd_b[w // wph][:, w % wph, :]
                    .unsqueeze(2)
                    .broadcast_to([TP, NG, TP]),
                )
                nc.sync.dma_start(
                    out=Dw[TP : TP + 1, :, :],
                    in_=mrT_sb[NG * (w % wph) : NG * (w % wph) + NG, w // wph, :],
                )
                D_w[w] = Dw

            def stage_a(w):
                # skip rows in device token order (k, r); host pre-permuted
                skip_sb = pskip.tile([WTOK, DIM], F32, tag="skip")
                nc.sync.dma_start(out=skip_sb, in_=ski[w])
                skip_w[w] = skip_sb
                xw, Dw = x_w[w], D_w[w]
                # transpose-normalize to [d, t'] on PE (raw bf16 x stationary,
                # D_w moving): x^T column c = k*96 + p <-> token t = 4p + k
                xT_sb = pxt.tile([128, 3, T], BF16, tag="xT")
                for i in range(3):
                    tp = pp.tile([128, 512], F32, tag="pj")
                    for k in range(KK):
                        nc.tensor.matmul(
                            tp[:, k * TP : (k + 1) * TP],
                            lhsT=xw[0 : TP + 1, i, k, :],
                            rhs=Dw[:, KK * i + k, :],
                            start=True, stop=True,
                        )
                    nc.scalar.copy(out=xT_sb[:, i, :], in_=tp[:, 0:T])
                # projections: q, k -> [hd, t]
                qT_sb = pqkv.tile([HD, T], BF16, tag="qT")
                kT_sb = pqkv.tile([HD, T], BF16, tag="kT")
                for i, (w_sb, b_sb, has_b, dst) in enumerate(
                    ((wq_sb, bq_sb, qbias, qT_sb), (wk_sb, bk_sb, kbias, kT_sb))
                ):
                    ppj = pp.tile([HD, T], F32, tag="pj")
                    nc.tensor.matmul(
                        ppj, lhsT=w_sb, rhs=xT_sb[:, i, :], start=True, stop=True
                    )
                    if has_b:
                        nc.tensor.matmul(
                            ppj, lhsT=b_sb, rhs=ones1, start=False, stop=True,
                            skip_group_check=True,
                        )
                    nc.vector.tensor_copy(dst, ppj)
                # v -> [t, hd] (token-major, the AV stationary operand)
                pv = pp.tile([128, TT, HD], F32, tag="pj")
                for j in range(TT):
                    nc.tensor.matmul(
                        pv[:, j, :],
                        lhsT=xT_sb[:, 2, j * 128 : (j + 1) * 128],
                        rhs=wv_sb, start=True, stop=True,
                    )
                vh_sb = pqkv.tile([128, TT, HD], BF16, tag="vh")
                nc.vector.tensor_copy(vh_sb, pv)
                qT_w[w], kT_w[w], vh_w[w] = qT_sb, kT_sb, vh_sb

            def stage_cd(cw, dw):
                # Interleaved emission: window cw's dot+exp (stage C) with
                # window dw = cw-1's s/av (stage D). The dot PSUM has one
                # 4-bank ring slot, so dot(kt+1) blocks the strict PE FIFO
                # until exp(kt) drains it; the s/av matmuls of the previous
                # window slotted between dot groups keep the PE fed through
                # those waits (and the HAM activity monitor warm).
                if cw is not None:
                    qT_sb, kT_sb = qT_w[cw], kT_w[cw]
                    expT_sb = pexp.tile([128, TT, HEADS, T], BF16, tag="expT")
                    expT_w[cw] = expT_sb
                if dw is not None:
                    expT_d, vh_sb = expT_w[dw], vh_w[dw]
                    s_ps = pssav.tile([HD, T], F32, tag="s")
                    av_ps = pssav.tile([HD, T], F32, tag="av")
                for kt in range(TT):
                    if cw is not None:
                        # 4 heads row-tiled (rows 32h) -> concurrent on PE;
                        # ONE Exp per kt (512-col spacing keeps each head's
                        # output inside one PSUM bank)
                        dt = psdot.tile([128, HEADS, 512], F32, tag="dot")
                        for h in range(HEADS):
                            nc.tensor.matmul(
                                dt[:, h, 0:T],
                                lhsT=kT_sb[32 * h : 32 * (h + 1), kt * 128 : (kt + 1) * 128],
                                rhs=qT_sb[32 * h : 32 * (h + 1), :],
                                start=True, stop=True,
                                tile_position=(32 * h, 0),
                            )
                        nc.scalar.activation(
                            out=expT_sb[:, kt, :, :],
                            in_=dt[:, :, 0:T],
                            func=mybir.ActivationFunctionType.Exp,
                        )
                    if dw is not None:
                        # 4 heads' s then av back-to-back: distinct col
                        # groups run concurrently (~2.4x measured for 3-col)
                        for h in range(HEADS):
                            nc.tensor.matmul(
                                s_ps[32 * h : 32 * (h + 1), :],
                                lhsT=ones32, rhs=expT_d[:, kt, h, :],
                                start=(kt == 0), stop=(kt == TT - 1),
                                tile_position=(0, 32 * h), skip_group_check=True,
                            )
                        for h in range(HEADS):
                            nc.tensor.matmul(
                                av_ps[32 * h : 32 * (h + 1), :],
                                lhsT=vh_sb[:, kt, 32 * h : 32 * (h + 1)],
                                rhs=expT_d[:, kt, h, :],
                                start=(kt == 0), stop=(kt == TT - 1),
                                tile_position=(0, 32 * h), skip_group_check=True,
                            )
                if dw is not None:
                    # 1/s = Exp(-Ln(s)) on ACT (both in the softmax-Exp
                    # table set, no thrash; frees the DVE's 8-cycle divide)
                    ls_sb = prs.tile([HD, T], F32, tag="ls")
                    nc.scalar.activation(
                        out=ls_sb, in_=s_ps,
                        func=mybir.ActivationFunctionType.Ln,
                    )
                    rs_sb = prs.tile([HD, T], F32, tag="rs")
                    nc.scalar.activation(
                        out=rs_sb, in_=ls_sb,
                        func=mybir.ActivationFunctionType.Exp, scale=-1.0,
                    )
                    aT_sb = pat.tile([HD, T], BF16, tag="aT")
                    nc.vector.tensor_mul(aT_sb, av_ps, rs_sb)
                    aT_w[dw] = aT_sb

            def stage_e(w):
                aT_sb = aT_w[w]
                # out proj all 6 agents in one matmul; mean over agents as a
                # strided DVE reduce: q col c = k*96 + n*16 + r (u = 4r + k).
                # z occupies cols 0..383 of a pj slot; the transposed result
                # is carved into the same bank at cols 384..511.
                # (moving the agent-mean BEFORE the z matmul was tried and
                # measured 5.6us SLOWER: the reduce lands on the aT->z
                # critical path instead of overlapping downstream work.)
                zz = pp.tile([DIM, 512], F32, tag="pj")
                nc.tensor.matmul(
                    zz[:, 0:T], lhsT=wp_sb, rhs=aT_sb, start=True, stop=True
                )
                zT_sb = ptail.tile([DIM, WTOK], F32, tag="zT")
                nc.vector.tensor_reduce(
                    out=zT_sb.rearrange("p (k r) -> p k r", k=KK),
                    in_=zz[:, 0:T].rearrange("p (k n r) -> p k r n", k=KK, n=NAG),
                    axis=mybir.AxisListType.X,
                    op=mybir.AluOpType.add,
                )
                nc.tensor.transpose(
                    out=zz[0:WTOK, T : T + DIM], in_=zT_sb,
                    identity=identf[:, :],
                )
                out_sb = ptail.tile([WTOK, DIM], F32, tag="osb")
                nc.vector.tensor_add(out_sb, zz[0:WTOK, T : T + DIM], skip_w[w])
                # store in device partition order; host un-permutes.
                # On the Pool/SWDGE queue, NOT sync: a store waits ~10us at
                # the queue head for out_sb (trace: wait=10416 once per step)
                # and the strict FIFO would head-of-line block the next
                # windows' D_w row DMAs + skip loads behind it.
                nc.gpsimd.dma_start(out=outo[w], in_=out_sb)

            # stats for the first half up front, second half interleaved into
            # the pipeline (keeps the DVE FIFO free of not-yet-loaded waits)
            for w in range(wph):
                phase1_stats(w)
            combine_half(0)
            for step in range(n_win + 4):
                if wph + step < n_win:
                    phase1_stats(wph + step)
                if step == wph:
                    combine_half(1)
                if step < n_win:
                    stage_n(step)
                if 0 <= step - 1 < n_win:
                    stage_a(step - 1)
                cw = step - 2 if 0 <= step - 2 < n_win else None
                dw = step - 3 if 0 <= step - 3 < n_win else None
                if cw is not None or dw is not None:
                    stage_cd(cw, dw)
                if 0 <= step - 4 < n_win:
                    stage_e(step - 4)

    return nc


def _split_multiwaits(nc, limit=1):
    """The staged walrus build rejects instructions carrying more than one
    sync-wait condition. Tile attaches several to some instructions (and the
    kernel-tail drain); peel the extras onto preceding engine NoOps. HW-only:
    CoreSim's sem bookkeeping rejects the injected NoOps."""
    for f in nc.m.functions:
        for bb in f.blocks:
            new_list = []
            for inst in bb.instructions:
                si = getattr(inst, "sync_info", None)
                waits = list(si.on_wait) if si is not None and si.on_wait else []
                if len(waits) > limit:
                    extra, keep = waits[:-limit], waits[-limit:]
                    for j in range(0, len(extra), limit):
                        nop = mybir.InstNoOp(
                            name=nc.get_next_instruction_name(),
                            engine=inst.engine,
                            ins=[],
                            outs=[],
                            sync_info=mybir.SyncInfo(
                                on_wait=extra[j : j + limit], on_update=[]
                            ),
                        )
                        new_list.append(nop)
                    si.on_wait = keep
                new_list.append(inst)
            if len(new_list) != len(bb.instructions):
                bb.instructions = new_list
    return nc


def _prep(inputs):
    """Host-side constant folding + window gather + device-layout pack +
    shard. Returns (in_maps, qbias, kbias)."""
    f32 = np.float32
    q = np.asarray(inputs["q"], f32)
    k = np.asarray(inputs["k"], f32)
    v = np.asarray(inputs["v"], f32)
    skip = np.asarray(inputs["skip"], f32)
    gate = np.asarray(inputs["head_gate"], f32)
    lnqw, lnqb = np.asarray(inputs["ln_q_w"], f32), np.asarray(inputs["ln_q_b"], f32)
    lnkw, lnkb = np.asarray(inputs["ln_k_w"], f32), np.asarray(inputs["ln_k_b"], f32)
    lnvw, lnvb = np.asarray(inputs["ln_v_w"], f32), np.asarray(inputs["ln_v_b"], f32)
    wq, bq = np.asarray(inputs["wq"], f32), np.asarray(inputs["bq"], f32)
    wk, bk = np.asarray(inputs["wk"], f32), np.asarray(inputs["bk"], f32)
    wv, bv = np.asarray(inputs["wv"], f32), np.asarray(inputs["bv"], f32)
    wp, bp = np.asarray(inputs["wp"], f32), np.asarray(inputs["bp"], f32)

    # fold LN affine into the projections; fold softmax scale + head_gate
    # into the q side (dot*gate == (qh*gate).kh); fold sqrt(DIM) into all
    # three (device rstd_raw = rstd_true/sqrt(DIM))
    rdim = np.sqrt(np.float32(DIM))
    colscale = np.repeat(gate * SCALE, DHEAD)          # [HD]
    wq_f = (lnqw[:, None] * wq) * colscale[None, :] * rdim
    bq_f = lnqb @ wq * colscale + bq * colscale
    wk_f = lnkw[:, None] * wk * rdim
    bk_f = lnkb @ wk + bk
    wv_f = lnvw[:, None] * wv * rdim
    bv_f = lnvb @ wv + bv
    wp_f = wp / NAG
    # constant v offset passes straight through attention (softmax sums to 1)
    skip_c = bv_f @ wp + bp                             # [DIM]

    qbias = bool(np.any(bq_f != 0))
    kbias = bool(np.any(bk_f != 0))

    def windows(t):
        return t.transpose(0, 2, 3, 1, 4, 5, 6).reshape(NWIN, T, DIM)

    bf = ml_dtypes.bfloat16
    qkvw = np.stack([windows(q), windows(k), windows(v)], axis=1)
    # device layout [win, p, i, k, d]: token t = 4p + k, 3KB partition lines.
    # Rows 96..127 padded host-side: row 96 = 1.0 (the -mu*rstd matmul row),
    # rows 97..127 = 0 — no device-side memsets needed.
    qkv_dev = np.zeros((NWIN, 128, 3, KK, DIM), dtype=bf)
    qkv_dev[:, :TP] = (
        qkvw.reshape(NWIN, 3, TP, KK, DIM).transpose(0, 2, 1, 3, 4).astype(bf)
    )
    qkv_dev[:, TP] = np.ones((3, KK, DIM), dtype=bf)
    qkv_dev = np.ascontiguousarray(qkv_dev)
    skw = (skip + skip_c).reshape(NWIN, WTOK, DIM)
    # device partition order (k, r) for output token u = 4r + k
    sk_dev = np.ascontiguousarray(
        skw.reshape(NWIN, WTOK // KK, KK, DIM).transpose(0, 2, 1, 3)
        .reshape(NWIN, WTOK, DIM)
    )

    wq_b = np.ascontiguousarray(wq_f.astype(bf))
    wk_b = np.ascontiguousarray(wk_f.astype(bf))
    wv_b = np.ascontiguousarray(wv_f.astype(bf))
    wp_b = np.ascontiguousarray(wp_f.astype(bf))

    in_maps = []
    for c in range(N_CORES):
        sl = slice(c * WPC, (c + 1) * WPC)
        in_maps.append(
            {
                "qkvin": qkv_dev[sl],
                "skin": sk_dev[sl],
                "wq": wq_b,
                "wk": wk_b,
                "wv": wv_b,
                "wp": wp_b,
                "bq": np.ascontiguousarray(bq_f[None, :]),
                "bk": np.ascontiguousarray(bk_f[None, :]),
            }
        )
    return in_maps, qbias, kbias


_BUILD_CACHE = {}


def _trace_available():
    try:
        from antenv.axon_hooks import get_axon_ntff_profile_hook  # noqa: F401

        return get_axon_ntff_profile_hook() is not None
    except Exception:
        return False


def run_sharded(in_maps, qbias, kbias, iters=1, trace=False):
    key = (qbias, kbias, iters)
    if key not in _BUILD_CACHE:
        # wait-splitting is for the walrus compiler only; CoreSim paths use
        # build_nc directly without it
        _BUILD_CACHE[key] = _split_multiwaits(
            build_nc(WPC, qbias=qbias, kbias=kbias, iters=iters)
        )
    nc = _BUILD_CACHE[key]
    return run_bass_kernel_spmd(
        nc, in_maps, core_ids=list(range(N_CORES)), trace=trace,
    )


def kernel(**inputs) -> np.ndarray:
    in_maps, qbias, kbias = _prep(inputs)
    trace = bool(int(os.environ.get("KERNEL_TRACE", "0"))) and _trace_available()
    res = run_sharded(in_maps, qbias, kbias, iters=1, trace=trace)
    if trace and res.exec_time_ns is not None:
        kernel.last_exec_time_ns = res.exec_time_ns
        kernel.last_trace = res.instructions_and_trace
    out = np.concatenate([r["out"] for r in res.results], axis=0)  # [128,64,128]
    # device partition order (k, r) -> token order u = 4r + k
    out = (
        out.reshape(NWIN, KK, WTOK // KK, DIM)
        .transpose(0, 2, 1, 3)
        .reshape(NWIN, WTOK, DIM)
    )
    out = out.reshape(B, X, Y, W1, W2, DIM)
    return np.ascontiguousarray(out.astype(np.float32))
